# revision 1
# baseline (speedup 1.0000x reference)
"""Chamfer distance kernel for Trainium2 (8 NeuronCores).

Problem: xyz1, xyz2: [4, 8192, 3] f32. Outputs (dist1, dist2, idx1, idx2):
squared NN distance + int32 argmin in both directions per batch.

Sharding: 8 cores = 4 batches x 2 directions. Core 2b handles direction
xyz1[b]->xyz2[b]; core 2b+1 handles xyz2[b]->xyz1[b]. Each core brute-forces
8192 queries x 8192 targets.

Device algorithm (per core):
  - Host precomputes a K=30 bf16 "split lift" for queries and targets such
    that  sum_k QL[k,m] * TL[k,n]  ~=  -d(q_m, t_n)  to ~1e-6 abs error
    (3-way bf16 hi/mid/lo split of the standard 5-term distance lift;
    6 cross-term blocks of 5 rows each = 30 rows, still one PE pass).
  - PE: for each 128-query row-tile (64 tiles), 16 matmuls of [30,128]^T x
    [30,512] accumulate -d into PSUM.
  - ACT: copies PSUM -> SBUF dbuf [128, 8192] (f32).
  - DVE: max (top-8 values) + max_index (their indices) per row-tile.
    max_index for tile i-1 is issued after max of tile i to give the
    same-engine RAW hazard ~8us of spacing.
  - Outputs: top-8 values + indices per query.

Host then reranks the <=8 candidates per query with exact fp32 (x-y)^2
arithmetic (bit-identical to the CPU XLA reference), picking the smallest
index on ties. Candidate sets provably contain the true argmin because the
device approximation error (~2e-5) is far below the typical value gap to the
8th-best neighbor.
"""
import numpy as np
import ml_dtypes

import concourse.bass as bass
import concourse.mybir as mybir
from concourse.bass_utils import run_bass_kernel_spmd

B = 4
N = 8192          # queries per core (= points per cloud)
M = 8192          # targets per core
CH = 512          # matmul chunk (free dim)
NCH = M // CH     # 16 chunks per row-tile
RT = 128          # row-tile (queries per tile)
NRT = N // RT     # 64 row-tiles
KLIFT = 30        # 6 blocks x 5 lift rows
GRP = 4           # chunks per PSUM tensor (4 banks)
NGRP = NCH // GRP

_NC_CACHE = {}


def _gen_kernel():
    """Raw-bass kernel for one core: 8192x8192 -d matmul + top8/argmax."""
    nc = bass.Bass()
    qlift = nc.dram_tensor("qlift", [KLIFT, N], mybir.dt.bfloat16, kind="ExternalInput")
    tlift = nc.dram_tensor("tlift", [KLIFT, M], mybir.dt.bfloat16, kind="ExternalInput")
    vals_out = nc.dram_tensor("vals", [RT, NRT * 8], mybir.dt.float32, kind="ExternalOutput")
    idx_out = nc.dram_tensor("idx", [RT, NRT * 8], mybir.dt.uint32, kind="ExternalOutput")

    import contextlib
    with (
        nc.sbuf_tensor([KLIFT, N], mybir.dt.bfloat16) as ql_sb,
        nc.sbuf_tensor([KLIFT, M], mybir.dt.bfloat16) as tl_sb,
        nc.sbuf_tensor([RT, 2, M], mybir.dt.float32) as dbuf,
        nc.sbuf_tensor([RT, NRT * 8], mybir.dt.float32) as vals_sb,
        nc.sbuf_tensor([RT, NRT * 8], mybir.dt.uint32) as idx_sb,
        nc.semaphore() as s_in,
        nc.semaphore() as s_mm,
        nc.semaphore() as s_cp,
        nc.semaphore() as s_dv,
        nc.semaphore() as s_out,
        contextlib.ExitStack() as st,
    ):
        ps = [st.enter_context(nc.psum_tensor(f"ps{t}", [RT, GRP, CH], mybir.dt.float32))
              for t in range(2)]
        with nc.Block() as block:

            @block.sync
            def _(sync):
                sync.dma_start(ql_sb[:, :], qlift[:, :]).then_inc(s_in, 16)
                sync.dma_start(tl_sb[:, :], tlift[:, :]).then_inc(s_in, 16)
                sync.wait_ge(s_dv, 2 * NRT)
                sync.dma_start(vals_out[:, :], vals_sb[:, :]).then_inc(s_out, 16)
                sync.dma_start(idx_out[:, :], idx_sb[:, :]).then_inc(s_out, 16)
                sync.wait_ge(s_out, 32)

            @block.tensor
            def _(tensor):
                tensor.wait_ge(s_in, 32)
                g = 0  # global group counter
                for i in range(NRT):
                    lhsT = ql_sb[:, i * RT : (i + 1) * RT]
                    for j in range(NGRP):
                        t = g % 2
                        if g >= 2:
                            # PSUM group reuse: ACT must have copied group g-2
                            tensor.wait_ge(s_cp, g - 1)
                        for k in range(GRP):
                            c = j * GRP + k
                            tensor.matmul(
                                ps[t][:, k, :], lhsT,
                                tl_sb[:, c * CH : (c + 1) * CH],
                                start=True, stop=True,
                            ).then_inc(s_mm, 1)
                        g += 1

            @block.scalar
            def _(scalar):
                g = 0
                for i in range(NRT):
                    buf = i % 2
                    for j in range(NGRP):
                        t = g % 2
                        scalar.wait_ge(s_mm, (g + 1) * GRP)
                        if j == 0 and i >= 2:
                            # dbuf[buf] free once max_index(i-2) is done
                            scalar.wait_ge(s_dv, 2 * i - 1)
                        scalar.copy(
                            dbuf[:, buf, j * GRP * CH : (j + 1) * GRP * CH],
                            ps[t][:, :, :],
                        ).then_inc(s_cp, 1)
                        g += 1

            @block.vector
            def _(vector):
                for i in range(NRT):
                    buf = i % 2
                    vector.wait_ge(s_cp, (i + 1) * NGRP)
                    vector.max(
                        out=vals_sb[:, i * 8 : (i + 1) * 8],
                        in_=dbuf[:, buf, :],
                    ).then_inc(s_dv, 1)
                    if i >= 1:
                        pb = (i - 1) % 2
                        vector.max_index(
                            out=idx_sb[:, (i - 1) * 8 : i * 8],
                            in_max=vals_sb[:, (i - 1) * 8 : i * 8],
                            in_values=dbuf[:, pb, :],
                        ).then_inc(s_dv, 1)
                # trailing max_index for the last tile (RAW spacing via nops)
                for _ in range(24):
                    vector.engine_nop()
                vector.max_index(
                    out=idx_sb[:, (NRT - 1) * 8 : NRT * 8],
                    in_max=vals_sb[:, (NRT - 1) * 8 : NRT * 8],
                    in_values=dbuf[:, (NRT - 1) % 2, :],
                ).then_inc(s_dv, 1)
    return nc


def _split3(a):
    """3-way bf16 split: a ~= h + m + l."""
    a = a.astype(np.float32)
    h = a.astype(ml_dtypes.bfloat16)
    r = a - h.astype(np.float32)
    m = r.astype(ml_dtypes.bfloat16)
    l = (r - m.astype(np.float32)).astype(ml_dtypes.bfloat16)
    return h, m, l


def _lift_q(Q):
    """[n,3] -> [5,n] f32 rows: 2qx, 2qy, 2qz, -|q|^2, 1."""
    n = Q.shape[0]
    return np.stack(
        [2 * Q[:, 0], 2 * Q[:, 1], 2 * Q[:, 2],
         -(Q * Q).sum(-1, dtype=np.float32), np.ones(n, np.float32)], 0
    ).astype(np.float32)


def _lift_t(T):
    n = T.shape[0]
    return np.stack(
        [T[:, 0], T[:, 1], T[:, 2], np.ones(n, np.float32),
         -(T * T).sum(-1, dtype=np.float32)], 0
    ).astype(np.float32)


def _split_lift(Lq, Lt):
    """Build K=30 bf16 operands whose inner product reproduces Lq.T @ Lt
    to ~1e-6: blocks (qh,th), (qh,tm), (qm,th), (qh,tl), (qm,tm), (ql,th)."""
    qh, qm, ql = _split3(Lq)
    th, tm, tl = _split3(Lt)
    QL = np.concatenate([qh, qh, qm, qh, qm, ql], 0)
    TL = np.concatenate([th, tm, th, tl, tm, th], 0)
    return np.ascontiguousarray(QL), np.ascontiguousarray(TL)


def _host_rerank(cand, Q, T):
    """Exact fp32 rerank of <=8 candidates per query.

    cand: [n, 8] uint32 candidate target indices (garbage allowed)
    Returns (dist [n] f32, idx [n] int32) matching f32 argmin semantics
    (smallest index on exact ties).
    """
    n = cand.shape[0]
    ci = cand.astype(np.int64)
    invalid = ci >= T.shape[0]
    ci_safe = np.where(invalid, 0, ci)
    t = T[ci_safe]                      # [n, 8, 3]
    q = Q[:, None, :]                   # [n, 1, 3]
    dx = q[..., 0] - t[..., 0]
    dy = q[..., 1] - t[..., 1]
    dz = q[..., 2] - t[..., 2]
    d = (dx * dx + dy * dy) + dz * dz   # exact f32, same order as reference
    d = np.where(invalid, np.float32(np.inf), d)
    # lexicographic min by (d, idx)
    order = np.lexsort((ci_safe, d), axis=-1)
    k = order[:, 0]
    rows = np.arange(n)
    return d[rows, k].astype(np.float32), ci_safe[rows, k].astype(np.int32)


def kernel(xyz1, xyz2):
    xyz1 = np.ascontiguousarray(np.asarray(xyz1, dtype=np.float32))
    xyz2 = np.ascontiguousarray(np.asarray(xyz2, dtype=np.float32))
    assert xyz1.shape == (B, N, 3) and xyz2.shape == (B, M, 3)

    if "nc" not in _NC_CACHE:
        _NC_CACHE["nc"] = _gen_kernel()
    nc = _NC_CACHE["nc"]

    # per-core inputs: core 2b -> (Q=xyz1[b], T=xyz2[b]); core 2b+1 swapped
    in_maps = []
    QT = []
    for b in range(B):
        for d in range(2):
            Q, T = (xyz1[b], xyz2[b]) if d == 0 else (xyz2[b], xyz1[b])
            QL, TL = _split_lift(_lift_q(Q), _lift_t(T))
            in_maps.append({"qlift": QL.astype(ml_dtypes.bfloat16),
                            "tlift": TL.astype(ml_dtypes.bfloat16)})
            QT.append((Q, T))

    res = run_bass_kernel_spmd(nc, in_maps, core_ids=list(range(8)))

    dist1 = np.empty((B, N), np.float32)
    dist2 = np.empty((B, M), np.float32)
    idx1 = np.empty((B, N), np.int32)
    idx2 = np.empty((B, M), np.int32)
    for core in range(8):
        b, d = divmod(core, 2)
        r = res.results[core]
        # [128, 64*8] -> [8192, 8]: query g = i*128 + p  ->  vals[p, i*8+k]
        cand = r["idx"].reshape(RT, NRT, 8).transpose(1, 0, 2).reshape(N, 8)
        Q, T = QT[core]
        dist, idx = _host_rerank(cand, Q, T)
        if d == 0:
            dist1[b], idx1[b] = dist, idx
        else:
            dist2[b], idx2[b] = dist, idx
    return dist1, dist2, idx1, idx2


# revision 15
# speedup vs baseline: 218.3762x; 218.3762x over previous
"""Chamfer distance kernel for Trainium2 (8 NeuronCores).

Inputs: xyz1, xyz2: [4, 8192, 3] f32. Outputs (dist1, dist2, idx1, idx2):
squared nearest-neighbor distances and int32 argmin indices in both
directions per batch, matching the fp32 reference exactly.

Sharding: 8 cores = 4 batches x 2 directions. Core 2b computes
xyz1[b]->xyz2[b] (dist1/idx1), core 2b+1 computes xyz2[b]->xyz1[b]
(dist2/idx2). Each core brute-forces 8192 queries x 8192 targets.

Per-core device algorithm:
  * Host precomputes a K=30 bf16 "split lift" such that
      sum_k QL[k,m] * TL[k,n]  ~=  -d(q_m, t_n)
    to ~2e-5 abs error: the standard 5-term distance lift
    [2q | -|q|^2 | 1] . [t | 1 | -|t|^2], with every f32 entry split
    3-way into bf16 (hi/mid/lo) and the 6 largest cross-term blocks
    stacked along the contraction dim. K=30 <= 32, so the matmul costs
    the same as K=5 but carries ~fp32 precision through the PSUM f32
    accumulator at full bf16 PE speed.
  * PE: per 128-query row-tile (64 tiles), 16 matmuls [30,128]^T x
    [30,512] write -d chunks into the 8 PSUM banks (2 groups of 4,
    double buffered).
  * ACT (scalar engine): casts each PSUM group to bf16 into an SBUF
    buffer (ring of 3 tiles). This is the throughput-limiting pass
    (1 elem/lane/cycle @ 1.2 GHz).
  * DVE (vector engine): lane-aligned in-place tree fold of the 8192
    bf16 values down to 512 lanes (4 tensor_tensor max ops, bf16 2x
    mode), then `max` (top-8 values) + `max_index` (their lanes).
    max_index of tile i-1 is issued between the fold and max of tile i
    so the engine's posted-write latency can never feed it stale data.
  * Outputs per query: top-8 folded lanes. Lane l encodes 16 possible
    target indices {c*512 + l}.

Host post-pass: expands the 8 lanes to 128 candidate indices, computes
exact fp32 (x-y)^2 distances for them (bit-identical op order to the
XLA CPU reference), picks the min with smallest-index tie-break. The
true argmin is provably in the candidate set for generic data because
the 8th-best gap vastly exceeds the lift error + bf16 rounding; a
detector recomputes any query with >=4 near-ties inside the bf16
comparison window by exact brute force (fires ~never on N(0,1) data,
keeps adversarial clustered inputs exact).

Measured: ~530 us device time per core (all cores run concurrently),
idx outputs exactly equal to the CPU reference, dist rel err ~5e-8
(last-ulp XLA FMA rounding differences only).
"""
import contextlib

import numpy as np
import ml_dtypes

import concourse.bass as bass
import concourse.mybir as mybir
from concourse.bass_utils import run_bass_kernel_spmd

B = 4             # batches
N = 8192          # queries per core (= points per cloud)
M = 8192          # targets per core
CH = 512          # matmul chunk (free dim; one PSUM bank)
NCH = M // CH     # 16 chunks per row-tile
RT = 128          # queries per row-tile
NRT = N // RT     # 64 row-tiles
KLIFT = 30        # 6 split blocks x 5 lift rows
GRP = 4           # chunks per PSUM group (4 banks)
NGRP = NCH // GRP

# on-device stopwatch calibration (gpsimd nop quantum), used by test.py
TIMER_QUANTUM_NS = 51457.0 / 60000.0  # ns per pool nop cycle (calibrated)
TIMER_NOP = 12000                     # pool cycles per watcher tick (~10.3us)

_NC_CACHE = {}


def _gen_kernel(repeat=1, timer_ticks=0):
    """Build the per-core bass program.

    repeat > 1 replays the whole compute (benchmarking).
    timer_ticks > 0 adds a gpsimd tick counter; output "tns" holds the
    tick count at compute completion (on-device stopwatch).
    """
    nc = bass.Bass()
    qlift = nc.dram_tensor("qlift", [KLIFT, N], mybir.dt.bfloat16, kind="ExternalInput")
    tlift = nc.dram_tensor("tlift", [KLIFT, M], mybir.dt.bfloat16, kind="ExternalInput")
    vals_out = nc.dram_tensor("vals", [RT, NRT * 8], mybir.dt.bfloat16, kind="ExternalOutput")
    idx_out = nc.dram_tensor("idx", [RT, NRT * 8], mybir.dt.uint32, kind="ExternalOutput")
    if timer_ticks:
        tns_out = nc.dram_tensor("tns", [1, 2], mybir.dt.int32, kind="ExternalOutput")

    with (
        nc.sbuf_tensor([KLIFT, N], mybir.dt.bfloat16) as ql_sb,
        nc.sbuf_tensor([KLIFT, M], mybir.dt.bfloat16) as tl_sb,
        nc.sbuf_tensor([RT, 3, M], mybir.dt.bfloat16) as dcast,
        nc.sbuf_tensor([RT, NRT * 8], mybir.dt.bfloat16) as vals_sb,
        nc.sbuf_tensor([RT, NRT * 8], mybir.dt.uint32) as idx_sb,
        nc.sbuf_tensor([1, 2], mybir.dt.int32) as cnt_sb,
        nc.semaphore() as s_in,
        nc.semaphore() as s_mm,
        nc.semaphore() as s_cp,
        nc.semaphore() as s_sc,
        nc.semaphore() as s_out,
        contextlib.ExitStack() as st,
    ):
        psall = st.enter_context(
            nc.psum_tensor("psall", [RT, 2 * GRP, CH], mybir.dt.float32))
        ps = [psall[:, 0:GRP, :], psall[:, GRP:2 * GRP, :]]
        with nc.Block() as block:

            @block.sync
            def _(sync):
                sync.dma_start(ql_sb[:, :], qlift[:, :]).then_inc(s_in, 16)
                sync.dma_start(tl_sb[:, :], tlift[:, :]).then_inc(s_in, 16)
                sync.wait_ge(s_sc, 2 * NRT * repeat)
                if timer_ticks:
                    sync.dma_start(tns_out[:, :], cnt_sb[:, :]).then_inc(s_out, 16)
                sync.dma_start(vals_out[:, :], vals_sb[:, :]).then_inc(s_out, 16)
                sync.dma_start(idx_out[:, :], idx_sb[:, :]).then_inc(s_out, 16)
                sync.wait_ge(s_out, 48 if timer_ticks else 32)

            if timer_ticks:
                @block.gpsimd
                def _(gpsimd):
                    gpsimd.wait_ge(s_in, 32)
                    with gpsimd.register("tk") as tk:
                        gpsimd.reg_mov(tk, 0)
                        for _ in range(timer_ticks):
                            gpsimd.nop(cycle_cnt=TIMER_NOP)
                            gpsimd.reg_add(tk, tk, 1)
                            gpsimd.reg_save(cnt_sb[0:1, 0:1], tk)

            @block.tensor
            def _(tensor):
                tensor.wait_ge(s_in, 32)
                g = 0  # global PSUM-group counter
                for gi in range(NRT * repeat):
                    i = gi % NRT
                    lhsT = ql_sb[:, i * RT : (i + 1) * RT]
                    for j in range(NGRP):
                        t = g % 2
                        if g >= 2:
                            # PSUM group reuse: ACT copied group g-2
                            tensor.wait_ge(s_cp, g - 1)
                        for k in range(GRP):
                            c = j * GRP + k
                            tensor.matmul(
                                ps[t][:, k, :], lhsT,
                                tl_sb[:, c * CH : (c + 1) * CH],
                                start=True, stop=True,
                            ).then_inc(s_mm, 1)
                        g += 1

            @block.scalar
            def _(scalar):
                g = 0
                for gi in range(NRT * repeat):
                    buf = gi % 3
                    for j in range(NGRP):
                        t = g % 2
                        scalar.wait_ge(s_mm, (g + 1) * GRP)
                        if j == 0 and gi >= 3:
                            # dcast[buf] free once max_index(gi-3) done
                            scalar.wait_ge(s_sc, 2 * gi - 4)
                        scalar.copy(
                            dcast[:, buf, j * GRP * CH : (j + 1) * GRP * CH],
                            ps[t][:, :, :],
                        ).then_inc(s_cp, 1)
                        g += 1

            @block.vector
            def _(vector):
                for gi in range(NRT * repeat):
                    i = gi % NRT
                    buf = gi % 3
                    d = dcast[:, buf, :]
                    vector.wait_ge(s_cp, (gi + 1) * NGRP)
                    # lane-aligned in-place tree fold: 8192 -> 512 lanes
                    w = M
                    while w > CH:
                        h = w // 2
                        vector.tensor_tensor(
                            d[:, 0:h], d[:, h:w], d[:, 0:h], mybir.AluOpType.max)
                        w = h
                    if gi >= 1:
                        pi = (gi - 1) % NRT
                        vector.max_index(
                            out=idx_sb[:, pi * 8 : (pi + 1) * 8],
                            in_max=vals_sb[:, pi * 8 : (pi + 1) * 8],
                            in_values=dcast[:, (gi - 1) % 3, 0:CH],
                        ).then_inc(s_sc, 1)
                    vector.max(
                        out=vals_sb[:, i * 8 : (i + 1) * 8], in_=d[:, 0:CH],
                    ).then_inc(s_sc, 1)
                # trailing max_index (nops give posted-write spacing)
                for _ in range(24):
                    vector.engine_nop()
                li = (NRT * repeat - 1) % NRT
                vector.max_index(
                    out=idx_sb[:, li * 8 : (li + 1) * 8],
                    in_max=vals_sb[:, li * 8 : (li + 1) * 8],
                    in_values=dcast[:, (NRT * repeat - 1) % 3, 0:CH],
                ).then_inc(s_sc, 1)
    return nc


def _split3(a):
    """3-way bf16 split: a ~= h + m + l (each bf16)."""
    a = a.astype(np.float32)
    h = a.astype(ml_dtypes.bfloat16)
    r = a - h.astype(np.float32)
    m = r.astype(ml_dtypes.bfloat16)
    l = (r - m.astype(np.float32)).astype(ml_dtypes.bfloat16)
    return h, m, l


def _lift_q(Q):
    """[n,3] -> [5,n] f32 rows: 2qx, 2qy, 2qz, -|q|^2, 1."""
    n = Q.shape[0]
    return np.stack(
        [2 * Q[:, 0], 2 * Q[:, 1], 2 * Q[:, 2],
         -(Q * Q).sum(-1, dtype=np.float32), np.ones(n, np.float32)], 0
    ).astype(np.float32)


def _lift_t(T):
    n = T.shape[0]
    return np.stack(
        [T[:, 0], T[:, 1], T[:, 2], np.ones(n, np.float32),
         -(T * T).sum(-1, dtype=np.float32)], 0
    ).astype(np.float32)


def _split_lift(Lq, Lt):
    """K=30 bf16 operand pair whose inner product reproduces Lq.T @ Lt to
    ~2e-5: blocks (qh,th), (qh,tm), (qm,th), (qh,tl), (qm,tm), (ql,th)."""
    qh, qm, ql = _split3(Lq)
    th, tm, tl = _split3(Lt)
    QL = np.concatenate([qh, qh, qm, qh, qm, ql], 0)
    TL = np.concatenate([th, tm, th, tl, tm, th], 0)
    return np.ascontiguousarray(QL), np.ascontiguousarray(TL)


def _host_rerank(cand, Q, T):
    """Exact fp32 rerank of candidate target indices per query.

    cand: [n, K] uint32 candidate indices (out-of-range values allowed).
    Returns (dist [n] f32, idx [n] int32) matching fp32 argmin semantics
    (smallest index on exact ties).

    Queries whose candidates show >=4 near-ties inside the bf16
    comparison window (where the device fold could have dropped the true
    argmin) are recomputed by exact brute force. Fires ~never on N(0,1)
    clouds; keeps adversarial clustered/duplicated inputs exact.
    """
    n = cand.shape[0]
    ci = cand.astype(np.int64)
    invalid = ci >= T.shape[0]
    ci_safe = np.where(invalid, 0, ci)
    t = T[ci_safe]                      # [n, K, 3]
    q = Q[:, None, :]
    dx = q[..., 0] - t[..., 0]
    dy = q[..., 1] - t[..., 1]
    dz = q[..., 2] - t[..., 2]
    d = (dx * dx + dy * dy) + dz * dz   # exact f32, same op order as reference
    d = np.where(invalid, np.float32(np.inf), d)
    order = np.lexsort((ci_safe, d), axis=-1)   # by (d, idx)
    k = order[:, 0]
    rows = np.arange(n)
    dist = d[rows, k].astype(np.float32)
    idx = ci_safe[rows, k].astype(np.int32)

    w = dist * np.float32(2 ** -6) + np.float32(1e-4) * np.maximum(dist, 1.0)
    near = (d <= (dist + w)[:, None]).sum(1)
    suspect = np.where((near >= 4) | invalid.any(1))[0]
    for s0 in range(0, len(suspect), 256):
        rows_s = suspect[s0:s0 + 256]
        qd = Q[rows_s][:, None, :] - T[None, :, :]
        sq = qd * qd
        dd = (sq[..., 0] + sq[..., 1]) + sq[..., 2]
        ii = np.argmin(dd, axis=1)
        idx[rows_s] = ii.astype(np.int32)
        dist[rows_s] = dd[np.arange(len(rows_s)), ii]
    return dist, idx


def kernel(xyz1, xyz2):
    xyz1 = np.ascontiguousarray(np.asarray(xyz1, dtype=np.float32))
    xyz2 = np.ascontiguousarray(np.asarray(xyz2, dtype=np.float32))
    assert xyz1.shape == (B, N, 3) and xyz2.shape == (B, M, 3)

    if "nc" not in _NC_CACHE:
        _NC_CACHE["nc"] = _gen_kernel()
    nc = _NC_CACHE["nc"]

    # per-core inputs: core 2b -> (Q=xyz1[b], T=xyz2[b]); core 2b+1 swapped
    in_maps = []
    QT = []
    for b in range(B):
        for d in range(2):
            Q, T = (xyz1[b], xyz2[b]) if d == 0 else (xyz2[b], xyz1[b])
            QL, TL = _split_lift(_lift_q(Q), _lift_t(T))
            in_maps.append({"qlift": QL.astype(ml_dtypes.bfloat16),
                            "tlift": TL.astype(ml_dtypes.bfloat16)})
            QT.append((Q, T))

    res = run_bass_kernel_spmd(nc, in_maps, core_ids=list(range(8)))

    dist1 = np.empty((B, N), np.float32)
    dist2 = np.empty((B, M), np.float32)
    idx1 = np.empty((B, N), np.int32)
    idx2 = np.empty((B, M), np.int32)
    chunk_off = (np.arange(NCH, dtype=np.int64) * CH)[None, :, None]  # [1,16,1]
    for core in range(8):
        b, d = divmod(core, 2)
        r = res.results[core]
        # [128, 64*8] -> [8192, 8]: query g = i*128 + p  ->  lanes[p, i*8+k]
        lanes = r["idx"].reshape(RT, NRT, 8).transpose(1, 0, 2).reshape(N, 8)
        # expand each winning lane to its 16 possible chunks
        cand = (lanes.astype(np.int64)[:, None, :] + chunk_off).reshape(N, NCH * 8)
        bad = lanes[:, None, :].repeat(NCH, 1).reshape(N, NCH * 8) >= CH
        cand = np.where(bad, np.int64(M), cand)  # invalid lanes (shouldn't happen)
        Q, T = QT[core]
        dist, idx = _host_rerank(cand.astype(np.uint32), Q, T)
        if d == 0:
            dist1[b], idx1[b] = dist, idx
        else:
            dist2[b], idx2[b] = dist, idx
    return dist1, dist2, idx1, idx2
